# revision 4
# baseline (speedup 1.0000x reference)
"""Trainium2 Bass kernel for GraphTransitionModel (GNN message passing).

Model (per batch element b, N=256 nodes):
  x[i]   = (obs[b,i], i/N)                              node features, 2-dim
  h1     = relu(A^T x_i + B^T x_j + a*w4 + b0)          messenger layer 1, 64
  h2     = relu(W1^T h1 + b1)                           64
  h3     = relu(W2^T h2 + b2)                           64
  m(i,j) = w3 . h3 + b3                                 scalar
  msg[i] = sum_j m(i,j)
  out    = MLP_updater([x_i, msg[i]])  (3->64->64->64->1)

Strategy: pure data parallel, 4 batch elements per core x 8 cores.

v2 over the 276us baseline: the PE-array weight residency scheme.
W1 lives in the diagonal 64x64 quadrants of the PE array, W2 in the
anti-diagonal quadrants (the j-half swap it causes is harmless: all
four msg partials are summed).  mm1/mm2 run as per-quadrant matmuls
(tile_position) whose legalize-inserted LDWEIGHTS are deleted
post-hoc, so the ~95ns/ldweights serial cost (4x per chunk in the
baseline = ~55us total) disappears.  mm3's w3s [128,2] stationary
overlays array cols 0-1 (fused ldweights kept) and a [128,2] strip
reload restores W1/W2 cols 0-1; the overlay runs every MM3_EVERY
chunks.  Setup-time first-layer matmuls clobber array rows 0-1 and
are followed by a [2,128] row restore.

Elementwise drains are split DVE/ACT at column CD per z-tile so both
engines carry ~1.5us/chunk; mm2/mm3 are split into range-aligned
matmuls so every matmul waits on exactly one producer.

The updater MLP runs in fp16 (HW-verified exact vs fp32 for these
magnitudes; bf16 was 2.8e-2 rel err) so its tail shrinks ~18us->~5us.
"""

import os
import sys
import numpy as np

sys.path.insert(0, "/opt/trn_rl_repo")

B, N, MID = 32, 256, 64
NCORES = 8
BPC = B // NCORES  # 4
HALF = N // 2  # 128 stacked j-iterations
JCH = 4
NCH = HALF // JCH  # 32 chunks per batch
FREE = JCH * N  # 1024
CD = 288  # DVE drain columns per z tile (rest on ACT)
MM3_EVERY = 2  # chunks per w3s overlay burst
FB = BPC * N  # 1024 updater free dim

# bf16 wpack16 column layout
C_W1Q = 0      # [128, 64] W1 twice (rows 0:64 / 64:128)
C_W2Q = 64     # [128, 64] W2 twice
C_W3S = 128    # [128, 2]
C_STRIP = 130  # [128, 2] cols 0-1 restore (W1 | W2)
C_ROW01 = 132  # [2, 128] rows 0-1 restore (W1 rows | W2 rows)
C_W0A = 260    # [2, 128] (A|A)
C_W0B = 388    # [2, 128] (B|B)
C_T16 = 516

# fp32 wpack column layout (biases only)
C_B1S = 0
C_B2S = 1
C_UB0 = 2
C_UB1 = 3
C_UB2 = 4
C_UB3 = 5
C_T32 = 6

# fp16 wpackh column layout (updater weights)
C_UW0 = 0
C_UW1 = 64
C_UW2 = 128
C_UW3 = 192
C_TH = 193

N_WARM = 40  # dummy warmup matmuls


def _build_bass():
    import concourse.bass as bass
    import concourse.bacc as bacc
    import concourse.tile as tile
    from concourse import mybir

    f32 = mybir.dt.float32
    bf16 = mybir.dt.bfloat16
    fp16 = mybir.dt.float16
    AF = mybir.ActivationFunctionType
    ALU = mybir.AluOpType

    nc = bacc.Bacc("TRN2", target_bir_lowering=False, num_devices=NCORES)

    wp_d = nc.declare_dram_parameter("wpack", [128, C_T32], f32, isOutput=False)
    wp16_d = nc.declare_dram_parameter("wpack16", [128, C_T16], bf16, isOutput=False)
    wph_d = nc.declare_dram_parameter("wpackh", [64, C_TH], fp16, isOutput=False)
    xT_d = nc.declare_dram_parameter("xT", [BPC, 2, N], f32, isOutput=False)
    ab0_d = nc.declare_dram_parameter("ab0", [BPC, MID, 1], f32, isOutput=False)
    out_d = nc.declare_dram_parameter("out", [BPC, N], f32, isOutput=True)

    marks = []  # matmul names whose legalize-LDWEIGHTS gets deleted

    with tile.TileContext(nc) as tc:
        with (
            tc.tile_pool(name="consts", bufs=1) as consts,
            tc.tile_pool(name="perb", bufs=2) as perb,
            tc.tile_pool(name="wk1", bufs=12) as wk1,
            tc.tile_pool(name="wk2", bufs=4) as wk2,
            tc.tile_pool(name="wk3", bufs=6) as wk3,
            tc.tile_pool(name="wkt", bufs=1) as wkt,
            tc.tile_pool(name="ps_z", bufs=3, space="PSUM") as ps_z,
            tc.tile_pool(name="ps_msg", bufs=1, space="PSUM") as ps_msg,
            tc.tile_pool(name="ps_small", bufs=1, space="PSUM") as ps_small,
        ):
            _last_pe = [None]

            def TMM(*a, **kw):
                m = nc.tensor.matmul(*a, **kw)
                if _last_pe[0] is not None:
                    bass._add_dep_helper(m.ins, _last_pe[0], sync=False,
                                         reason="pe-order")
                _last_pe[0] = m.ins
                return m

            def TLDW(*a, **kw):
                l = nc.tensor.ldweights(*a, **kw)
                if _last_pe[0] is not None:
                    bass._add_dep_helper(l.ins, _last_pe[0], sync=False,
                                         reason="pe-order")
                _last_pe[0] = l.ins
                return l
            # warmup: memset a tiny tile, then dummy 1-col matmuls keep the
            # PE busy through the const DMAs so HAM is at K=8/8 when the
            # first real matmul issues.
            dmt = consts.tile([1, 2], bf16, tag="dmt")
            nc.vector.memset(dmt[:], 0.0)
            psw = ps_small.tile([1, 1], f32, tag="pss")
            for _ in range(N_WARM):
                TMM(
                    psw[:], dmt[0:1, 0:1], dmt[0:1, 1:2],
                    start=True, stop=True, skip_group_check=True,
                )

            # const DMAs: tiny batch-0 inputs first, then bf16 pack
            # (setup-critical), then fp16/fp32 packs.
            uin = consts.tile([6, FB], f32, tag="uin")
            nc.sync.dma_start(out=uin[0:2, 0:N], in_=xT_d[0])
            ab0s0 = perb.tile([128, 1], f32, tag="ab0s")
            src_ab0 = ab0_d[0]
            nc.sync.dma_start(
                out=ab0s0[:],
                in_=bass.AP(
                    tensor=src_ab0.tensor,
                    offset=src_ab0.offset,
                    ap=[[0, 2]] + list(src_ab0.ap),
                ),
            )
            wp16 = consts.tile([128, C_T16], bf16, tag="wpack16")
            nc.sync.dma_start(out=wp16[:], in_=wp16_d[:])
            wph = consts.tile([64, C_TH], fp16, tag="wpackh")
            nc.sync.dma_start(out=wph[:], in_=wph_d[:])
            wp = consts.tile([128, C_T32], f32, tag="wpack")
            nc.sync.dma_start(out=wp[:], in_=wp_d[:])

            w1q = wp16[:, C_W1Q : C_W1Q + 64]
            w2q = wp16[:, C_W2Q : C_W2Q + 64]
            w3s = wp16[:, C_W3S : C_W3S + 2]
            strip = wp16[:, C_STRIP : C_STRIP + 2]
            row01 = wp16[0:2, C_ROW01 : C_ROW01 + 128]
            w0a16 = wp16[0:2, C_W0A : C_W0A + 128]
            w0b16 = wp16[0:2, C_W0B : C_W0B + 128]
            b1s = wp[:, C_B1S : C_B1S + 1]
            b2s = wp[:, C_B2S : C_B2S + 1]
            ub0 = wp[0:MID, C_UB0 : C_UB0 + 1]
            ub1 = wp[0:MID, C_UB1 : C_UB1 + 1]
            ub2 = wp[0:MID, C_UB2 : C_UB2 + 1]
            ub3 = wp[0:1, C_UB3 : C_UB3 + 1]
            uw0 = wph[0:6, C_UW0 : C_UW0 + MID]
            uw1 = wph[0:MID, C_UW1 : C_UW1 + MID]
            uw2 = wph[0:MID, C_UW2 : C_UW2 + MID]
            uw3 = wph[0:MID, C_UW3 : C_UW3 + 1]

            # resident quadrant loads (absorb the wp16 DMA wait):
            # W1 diag, W2 anti-diag.
            TLDW(w1q[0:64, :], tile_position=(0, 0))
            TLDW(w1q[64:128, :], tile_position=(64, 64))
            TLDW(w2q[0:64, :], tile_position=(0, 64))
            TLDW(w2q[64:128, :], tile_position=(64, 0))

            pend = {}

            def setup_s0(b):
                bcs = slice(b * N, (b + 1) * N)
                if b == 0:
                    ab0s = ab0s0
                else:
                    nc.sync.dma_start(out=uin[0:2, bcs], in_=xT_d[b])
                    ab0s = perb.tile([128, 1], f32, tag="ab0s")
                    src_ab = ab0_d[b]
                    ab_bcast = bass.AP(
                        tensor=src_ab.tensor,
                        offset=src_ab.offset,
                        ap=[[0, 2]] + list(src_ab.ap),
                    )
                    nc.sync.dma_start(out=ab0s[:], in_=ab_bcast)
                uin16 = perb.tile([2, N], bf16, tag="uin16")
                nc.scalar.copy(uin16[:], uin[0:2, bcs])
                pend[b] = [ab0s, uin16]

            def setup_s1(b):
                # psP's fused ldweights (w0a16, [2,128]) clobbers array rows
                # 0-1; restore them before the next body's matmuls.
                ab0s, uin16 = pend[b]
                psP = ps_small.tile([128, N], f32, tag="pss")
                TMM(psP[:], w0a16, uin16[:], start=True, stop=True)
                TLDW(row01)
                Pd = perb.tile([128, N], bf16, tag="Pd")
                nc.vector.tensor_copy(Pd[:], psP[:])
                pend[b].append(Pd)

            def setup_s2(b):
                ab0s, uin16, Pd = pend[b]
                psQ = ps_small.tile([128, N], f32, tag="pss")
                TMM(psQ[:], w0b16, uin16[:], start=True, stop=True)
                TLDW(row01)
                qtmp = perb.tile([128, N], f32, tag="qtmp")
                nc.scalar.activation(qtmp[:], psQ[:], AF.Identity, bias=ab0s)
                Qs = perb.tile([128, HALF], f32, tag="Qs")
                nc.sync.dma_start(out=Qs[0:MID, :], in_=qtmp[0:MID, 0:HALF])
                nc.sync.dma_start(out=Qs[MID:128, :], in_=qtmp[MID:128, HALF:N])
                pend[b].append(Qs)

            def setup_s3(b):
                ab0s, uin16, Pd, Qs = pend.pop(b)
                # DVE fence: Qs has two DMA writers
                Qs2 = perb.tile([128, HALF], f32, tag="Qs2")
                nc.vector.tensor_copy(Qs2[:], Qs[:])
                msg_ps = ps_msg.tile([2, 2 * N], f32, tag="msg")
                return (Pd, Qs2, msg_ps)

            def emit_setup(b):
                setup_s0(b)
                setup_s1(b)
                setup_s2(b)
                return setup_s3(b)

            # mm2/mm3 ranges: single-producer per matmul, and no matmul
            # output may cross a PSUM bank boundary (col 512)
            R2 = [(0, 512), (512, FREE)]
            HD = 608  # h3 DVE/ACT drain split
            R3 = [(0, 512, 0), (512, HD, 0), (HD, FREE, HD - 512)]

            T = BPC * NCH
            ctx = {0: emit_setup(0)}
            h1q, z2q, h2q, z3q, h3q = {}, {}, {}, {}, {}
            mm3_pend = []  # (x, h3 tile) awaiting a burst

            def mm3_burst():
                # w3s overlay: fused ldweights on the first matmul loads
                # w3s into array cols 0-1; strip reload restores W1/W2.
                first = True
                for x, h3t in mm3_pend:
                    bx, cx = divmod(x, NCH)
                    msg_x = ctx[bx][2]
                    for a, b_, mo in R3:
                        m = TMM(
                            msg_x[:, mo : mo + (b_ - a)], w3s, h3t[:, a:b_],
                            start=(cx == 0 and a == 0),
                            stop=(cx == NCH - 1 and a == HD),
                            skip_group_check=True,
                        )
                        if not first:
                            marks.append(m.ins.name)
                        first = False
                    if cx == NCH - 1:
                        bcx = slice(bx * N, (bx + 1) * N)
                        msg_sb = perb.tile([2, 2 * N], f32, tag="msg_sb")
                        nc.scalar.copy(msg_sb[:], msg_x[:])
                        nc.sync.dma_start(out=uin[2:4, bcx], in_=msg_sb[:, 0:N])
                        nc.sync.dma_start(
                            out=uin[4:6, bcx], in_=msg_sb[:, N : 2 * N]
                        )
                        del ctx[bx]
                mm3_pend.clear()
                TLDW(strip)

            for t in range(T + 4):
                # stage 4a: h3(t-3) drain split DVE/ACT
                if 3 <= t < T + 3:
                    x = t - 3
                    z3p = z3q.pop(x)
                    h3 = wk3.tile([128, FREE], bf16, tag="h3")
                    nc.vector.tensor_scalar(
                        h3[:, 0:HD], z3p[:, 0:HD], b2s, 0.0, ALU.add, ALU.max
                    )
                    nc.scalar.activation(
                        h3[:, HD:FREE], z3p[:, HD:FREE], AF.Relu, bias=b2s
                    )
                    h3q[x] = h3
                # stage 1: h1 quad(t) on DVE
                if t < T:
                    bq, cq = divmod(t, NCH)
                    Pd2, Qs2, _ = ctx[bq]
                    jb = cq * JCH
                    h1a = wk1.tile([128, 2 * N], bf16, tag="h1a")
                    nc.vector.tensor_scalar(
                        h1a[:, 0:N], Pd2[:], Qs2[:, jb : jb + 1],
                        0.0, ALU.add, ALU.max,
                    )
                    nc.vector.tensor_scalar(
                        h1a[:, N : 2 * N], Pd2[:], Qs2[:, jb + 1 : jb + 2],
                        0.0, ALU.add, ALU.max,
                    )
                    h1b = wk1.tile([128, 2 * N], bf16, tag="h1b")
                    nc.vector.tensor_scalar(
                        h1b[:, 0:N], Pd2[:], Qs2[:, jb + 2 : jb + 3],
                        0.0, ALU.add, ALU.max,
                    )
                    nc.vector.tensor_scalar(
                        h1b[:, N : 2 * N], Pd2[:], Qs2[:, jb + 3 : jb + 4],
                        0.0, ALU.add, ALU.max,
                    )
                    h1q[t] = (h1a, h1b)
                # stage 2a: mm1(t-1), resident diag quadrants
                if 1 <= t <= T:
                    x = t - 1
                    p1a, p1b = h1q.pop(x)
                    z2 = ps_z.tile([128, FREE], f32, tag="z")
                    for k, src in ((0, p1a), (512, p1b)):
                        m = TMM(
                            z2[0:64, k : k + 512], w1q[0:64, :], src[0:64, :],
                            start=True, stop=True, tile_position=(0, 0),
                            skip_group_check=True,
                        )
                        marks.append(m.ins.name)
                        m = TMM(
                            z2[64:128, k : k + 512], w1q[64:128, :],
                            src[64:128, :],
                            start=True, stop=True, tile_position=(64, 64),
                            skip_group_check=True,
                        )
                        marks.append(m.ins.name)
                    z2q[x] = z2
                # stage 3: mm2(t-2), resident anti-diag quadrants
                if 2 <= t <= T + 1:
                    x = t - 2
                    h2c = h2q.pop(x)
                    z3 = ps_z.tile([128, FREE], f32, tag="z")
                    for a, b_ in R2:
                        m = TMM(
                            z3[64:128, a:b_], w2q[0:64, :], h2c[0:64, a:b_],
                            start=True, stop=True, tile_position=(0, 64),
                            skip_group_check=True,
                        )
                        marks.append(m.ins.name)
                        m = TMM(
                            z3[0:64, a:b_], w2q[64:128, :], h2c[64:128, a:b_],
                            start=True, stop=True, tile_position=(64, 0),
                            skip_group_check=True,
                        )
                        marks.append(m.ins.name)
                    z3q[x] = z3
                # stage 4b: mm3 burst every MM3_EVERY chunks / at batch end
                if 3 <= t < T + 3:
                    x = t - 3
                    mm3_pend.append((x, h3q.pop(x)))
                    bx, cx = divmod(x, NCH)
                    if cx % MM3_EVERY == MM3_EVERY - 1 or cx == NCH - 1:
                        mm3_burst()
                # stage 2b: h2(t-1) drain split DVE/ACT
                if 1 <= t <= T:
                    x = t - 1
                    z2c = z2q.pop(x)
                    h2 = wk2.tile([128, FREE], bf16, tag="h2")
                    nc.scalar.activation(h2[:], z2c[:], AF.Relu, bias=b1s)
                    h2q[x] = h2
                # prefetch next batch's setup, one stage per body
                for off, stage in ((5, setup_s0), (4, setup_s1), (3, setup_s2)):
                    bn, rn = divmod(t + off, NCH)
                    if rn == 0 and 1 <= bn < BPC:
                        stage(bn)
                bn, rn = divmod(t + 2, NCH)
                if rn == 0 and 1 <= bn < BPC:
                    ctx[bn] = setup_s3(bn)

            # ---- updater MLP in fp16, all batches (free = FB) ----
            qs4 = [slice(0, 512), slice(512, 1024)]
            uinh = wkt.tile([6, FB], fp16, tag="uinh")
            nc.vector.tensor_copy(uinh[:], uin[:])

            def ulayer(wcol, src, bias, dst, last=False):
                # 4 slice matmuls (fused LDW on first only) + relu drains
                # alternating ACT/DVE.
                ps = ps_z.tile([MID, FB] if not last else [1, FB], f32, tag="z")
                for i, s in enumerate(qs4):
                    m = TMM(
                        ps[:, s], wcol, src[:, s], start=True, stop=True,
                        skip_group_check=True,
                    )
                    if i > 0:
                        marks.append(m.ins.name)
                for i, s in enumerate(qs4):
                    if last:
                        nc.scalar.activation(
                            dst[:, s], ps[:, s], AF.Identity, bias=bias
                        )
                    elif i % 2 == 0:
                        nc.scalar.activation(
                            dst[:, s], ps[:, s], AF.Relu, bias=bias
                        )
                    else:
                        nc.vector.tensor_scalar(
                            dst[:, s], ps[:, s], bias, 0.0, ALU.add, ALU.max
                        )

            t1 = wkt.tile([MID, FB], fp16, tag="t1")
            ulayer(uw0, uinh, ub0, t1)
            t2 = wkt.tile([MID, FB], fp16, tag="t2")
            ulayer(uw1, t1, ub1, t2)
            t3 = wkt.tile([MID, FB], fp16, tag="t3")
            ulayer(uw2, t2, ub2, t3)
            orow = consts.tile([1, FB], f32, tag="orow")
            ulayer(uw3, t3, ub3, orow, last=True)
            nc.sync.dma_start(out=out_d[:, :], in_=orow[:])

    # delete the legalize-inserted LDWEIGHTS of marked matmuls
    from concourse import mybir as mb

    markset = set(marks)
    removed = 0
    for blk in nc.main_func.blocks:
        insts = list(blk.instructions)
        drop = set()
        last_ldw = None
        for idx, i in enumerate(insts):
            if isinstance(i, mb.InstLdweights):
                last_ldw = idx
            elif isinstance(i, mb.InstMatmult):
                if i.name in markset and last_ldw is not None:
                    li = insts[last_ldw]
                    if not (li.has_wait() or li.has_update()):
                        drop.add(last_ldw)
                last_ldw = None
        if drop:
            blk.instructions = [
                i for idx, i in enumerate(insts) if idx not in drop
            ]
            removed += len(drop)
    print(f"deleted {removed}/{len(markset)} marked LDWs")

    nc.compile()
    return nc


def _host_inputs(inputs):
    import ml_dtypes

    g = lambda k: np.asarray(inputs[k], np.float32)
    obs, action = g("obs"), g("action")
    m_w0, m_b0, m_w1, m_b1 = g("m_w0"), g("m_b0"), g("m_w1"), g("m_b1")
    m_w2, m_b2, m_w3, m_b3 = g("m_w2"), g("m_b2"), g("m_w3"), g("m_b3")
    u_w0, u_b0, u_w1, u_b1 = g("u_w0"), g("u_b0"), g("u_w1"), g("u_b1")
    u_w2, u_b2, u_w3, u_b3 = g("u_w2"), g("u_b2"), g("u_w3"), g("u_b3")

    coor = np.arange(N, dtype=np.float32) / N
    xT = np.stack([obs, np.broadcast_to(coor, obs.shape)], axis=1)  # [B, 2, N]
    ab0 = (action[:, None] * m_w0[4] + m_b0).astype(np.float32)[..., None]

    wpack16 = np.zeros((128, C_T16), np.float32)
    wpack16[0:MID, C_W1Q : C_W1Q + MID] = m_w1
    wpack16[MID:128, C_W1Q : C_W1Q + MID] = m_w1
    wpack16[0:MID, C_W2Q : C_W2Q + MID] = m_w2
    wpack16[MID:128, C_W2Q : C_W2Q + MID] = m_w2
    wpack16[:MID, C_W3S] = m_w3[:, 0]
    wpack16[MID:, C_W3S + 1] = m_w3[:, 0]
    wpack16[0:MID, C_STRIP : C_STRIP + 2] = m_w1[:, 0:2]
    wpack16[MID:128, C_STRIP : C_STRIP + 2] = m_w2[:, 0:2]
    wpack16[0:2, C_ROW01 : C_ROW01 + MID] = m_w1[0:2, :]
    wpack16[0:2, C_ROW01 + MID : C_ROW01 + 128] = m_w2[0:2, :]
    wpack16[0:2, C_W0A : C_W0A + MID] = m_w0[0:2]
    wpack16[0:2, C_W0A + MID : C_W0A + 128] = m_w0[0:2]
    wpack16[0:2, C_W0B : C_W0B + MID] = m_w0[2:4]
    wpack16[0:2, C_W0B + MID : C_W0B + 128] = m_w0[2:4]
    wpack16 = wpack16.astype(ml_dtypes.bfloat16)

    wpack = np.zeros((128, C_T32), np.float32)
    wpack[:MID, C_B1S] = m_b1
    wpack[MID:, C_B1S] = m_b1
    wpack[:MID, C_B2S] = m_b2
    wpack[MID:, C_B2S] = m_b2
    # msg rows exclude the +N*b3 term; fold into the updater bias
    wpack[:MID, C_UB0] = u_b0 + N * float(m_b3[0]) * u_w0[2]
    wpack[:MID, C_UB1] = u_b1
    wpack[:MID, C_UB2] = u_b2
    wpack[0, C_UB3] = float(u_b3[0])

    wpackh = np.zeros((64, C_TH), np.float32)
    wpackh[0:2, C_UW0 : C_UW0 + MID] = u_w0[0:2]
    for _r in (2, 3, 4, 5):
        wpackh[_r, C_UW0 : C_UW0 + MID] = u_w0[2]
    wpackh[:, C_UW1 : C_UW1 + MID] = u_w1
    wpackh[:, C_UW2 : C_UW2 + MID] = u_w2
    wpackh[:, C_UW3] = u_w3[:, 0]
    wpackh = wpackh.astype(np.float16)

    in_maps = []
    for c in range(NCORES):
        sl = slice(c * BPC, (c + 1) * BPC)
        in_maps.append(
            dict(
                wpack=wpack,
                wpack16=wpack16,
                wpackh=wpackh,
                xT=np.ascontiguousarray(xT[sl]),
                ab0=np.ascontiguousarray(ab0[sl]),
            )
        )
    return in_maps


def kernel(**inputs) -> np.ndarray:
    in_maps = _host_inputs(inputs)

    from concourse.bass_utils import run_bass_kernel_spmd

    nc = _build_bass()
    res = run_bass_kernel_spmd(
        nc, in_maps, core_ids=list(range(NCORES)),
        trace=bool(int(os.environ.get("KERNEL_TRACE", "0"))),
    )
    out = np.concatenate([r["out"] for r in res.results], axis=0)  # [B, N]
    if res.exec_time_ns is not None:
        print(f"HW exec time: {res.exec_time_ns} ns")
        print(f"mean exec time: {res.mean_exec_time_ns} ns")
    return out.astype(np.float32)


if __name__ == "__main__":
    nc = _build_bass()
    print("bass build OK")
